# revision 15
# baseline (speedup 1.0000x reference)
"""Trainium2 Bass kernel for nn_Estor_concat (scatter_memory).

Math (exact reformulation of the reference):
  v_tag = (tag_emb @ Wv.T + bv) @ out_proj_w.T + out_proj_b        [T, H]
  W_eff[t, j] = sum_h v_tag[t, h] * ff1_w[j, t*H + h]              [T, H]
  counts[t, s] = #spans(tag t) covering s
  h1 = relu(W_eff.T @ counts + ff1_b)
  h2 = ff2 @ relu(h1) + ff2_b  (needed only for its per-position sumsq)
  x = [word_emb | h2]; LayerNorm folded into the output projection:
  out[l,s] = raw[l,s]*rstd[s] + (c1n[l]*mu[s] + d[l])*rstd[s] + c2[l]
  raw = lwg_we.T @ we + G.T @ relu  with  G = ff2.T @ lwg_h2  (host-folded)

Span coverage masks use a single fused compare per span:
  covered = (|iota - m| <= r)  with m=(s+e-1)/2, r=(e-s-1)/2
so each 128-span tile needs ONE [T,S] matmul (host-provided onehot lhsT),
half the passes of the ge/lt two-mask scheme and no rank-1 correction.

h1 runs in fp8 DoubleRow with a zero second slot in the lhsT (cost is
N-bound, so DR halves it even at K=17).  A short chain of warmup matmuls
pins the PE p-state clock early so real matmuls run at full rate.
Weight-only folds (lwg, G, c1n, d, c2, weffp8) happen on the host.
Sharding: pure data-parallel over batch (8 cores, 1 batch each).
"""

import ml_dtypes
import numpy as np

import concourse.bacc as bacc
import concourse.bass as bass
import concourse.mybir as mybir
import concourse.tile as tile
from concourse.bass_utils import run_bass_kernel_spmd

T, B, S, H = 16, 8, 512, 768
H2 = 384
NEW_H = H + H2          # 1152
NL = 33                 # num labels
EPS = 1e-12
NCORES = 8
P = 128
KC_H = H // 128         # 6 chunks of the hidden dim
KC_H2 = H2 // 128       # 3
MPR = 65                # pr psum rows: raw 0:33, zeros, sum row at 64
GSC = 16.0              # fp8 scale on G (relu emits r/(GSC*W8SC))
FSC = 32.0              # fp8 scale on ff2.T
W8SC = 64.0             # fp8 scale on weffp (h1 path)
W8 = 512                # padded pair width in the fp8 block (>= H2 + 65)
HS = S // 2

F32 = mybir.dt.float32
BF16 = mybir.dt.bfloat16
F16 = mybir.dt.float16
F8 = mybir.dt.float8e4
AF = mybir.ActivationFunctionType
ALU = mybir.AluOpType
DR = mybir.MatmulPerfMode.DoubleRow


def build_kernel(n_span_tiles: int):
    nc = bacc.Bacc(
        "TRN2",
        target_bir_lowering=False,
        debug=False,
        enable_asserts=True,
        num_devices=NCORES,
    )

    def inp(name, shape, dtype=F32):
        return nc.dram_tensor(name, list(shape), dtype, kind="ExternalInput").ap()

    NT = n_span_tiles
    CF16 = S + 2 * NT * T                 # iota | +/- onehot blocks
    CF32 = 2 * NT + KC_H2 + 2            # s | e | ff2b cols | mu_bias | c2
    CBF = KC_H * MPR + NL                # lwgwe chunks | [c1n;d] cols
    cf16 = inp("cf16", (P, CF16), F16)
    cf32 = inp("cf32", (P, CF32), F32)
    cbf = inp("cbf", (P, CBF), BF16)
    cf8 = inp("cf8", (P, KC_H2, 2, W8), F8)   # [ff2t8 | g65 | pad] per pair
    wf8 = inp("wf8", (T, 2, H), F8)           # weff8 | f1b row in slot 1
    we_t = inp("we_t", (P, KC_H, S), BF16)    # word_embedding[b].T chunked

    out = nc.dram_tensor("out", [NL, S], F32, kind="ExternalOutput").ap()

    O_M, O_R = 0, NT
    O_F2B = 2 * NT
    O_MUB = O_F2B + KC_H2
    O_C2 = O_MUB + 1
    OFF_W = KC_H * MPR                    # lwgwe chunks end / [c1n;d] start
    RSCL = 1.0 / (GSC * W8SC)             # relu -> r8 scale

    with tile.TileContext(nc) as tc:
        with (
            tc.tile_pool(name="singles", bufs=1) as singles,
            tc.tile_pool(name="masks", bufs=4) as masks,
            tc.tile_pool(name="work", bufs=10) as work,
            tc.tile_pool(name="ps_cnt", bufs=1, space="PSUM") as ps_cnt,
            tc.tile_pool(name="ps_big", bufs=4, space="PSUM") as ps_big,
            tc.tile_pool(name="ps_pr", bufs=1, space="PSUM") as ps_pr,
            tc.tile_pool(name="ps_ss", bufs=1, space="PSUM") as ps_ss,
            tc.tile_pool(name="ps_rb", bufs=1, space="PSUM") as ps_rb,
        ):
            # ---- small SBUF constants (DVE memsets, ready early) ----------
            ones_cb = singles.tile([1, 1], BF16)
            nc.vector.memset(ones_cb, 1.0)
            ones_s = singles.tile([1, S], BF16)
            nc.vector.memset(ones_s, 1.0)
            ones_row = singles.tile([1, NL], BF16)
            nc.vector.memset(ones_row, 1.0)
            neg1 = singles.tile([1, 1], BF16)
            nc.vector.memset(neg1, -1.0)
            inv_col = singles.tile([P, 1], BF16)
            nc.vector.memset(inv_col, 1.0 / NEW_H)
            zcol = singles.tile([P, 1], F32)
            nc.vector.memset(zcol, 0.0)
            eps_t = singles.tile([1, 1], F32)
            nc.vector.memset(eps_t, EPS)
            scratch = singles.tile([1, 1], F32)
            # force the sqrt_and_others act table set
            # (covers Sqrt + Relu + Identity + Square)
            nc.scalar.activation(out=scratch, in_=eps_t, func=AF.Sqrt)

            # ---- Pool memsets (counts8 zero fill + ones rows) -------------
            # f1b bias rides in DR slot 1 of partition 0 (wf8[0,1,:] = f1b):
            # counts8[0,1,:] = 1.0, all other slot-1 rows stay zero.
            counts8 = singles.tile([T, 2, S], F8)
            nc.gpsimd.memset(counts8, 0.0)
            nc.gpsimd.memset(counts8[0:1, 1:2, :], 1.0)
            mu_sb = singles.tile([2, S], BF16)
            nc.gpsimd.memset(mu_sb, 1.0)                     # row 1 stays ones

            # ---- input DMAs: SP = cf32|cbf|we, Act = cf16|cf8|wf8 ---------
            cf32_sb = singles.tile([P, CF32], F32)
            nc.sync.dma_start(out=cf32_sb, in_=cf32)
            cbf_sb = singles.tile([P, CBF], BF16)
            nc.sync.dma_start(out=cbf_sb, in_=cbf)
            we_sb = singles.tile([P, KC_H, S], BF16)
            nc.sync.dma_start(out=we_sb[:, 0:3, :], in_=we_t[:, 0:3, :])
            nc.sync.dma_start(out=we_sb[:, 3:6, :], in_=we_t[:, 3:6, :])
            cf16_sb = singles.tile([P, CF16], F16)
            nc.scalar.dma_start(out=cf16_sb, in_=cf16)
            cf8_sb = singles.tile([P, KC_H2, 2, W8], F8)
            nc.scalar.dma_start(out=cf8_sb, in_=cf8)
            wf8_sb = singles.tile([T, 2, H], F8)
            nc.scalar.dma_start(out=wf8_sb, in_=wf8)

            iota16 = cf16_sb[:, 0:S]

            # ---- PE warmup: pin pe_busy_start early so the p-state clock
            # reaches full rate by the time real matmuls flow.
            for w in range(3):
                warm = ps_big.tile([1, S], F32, tag="big", name=f"warm{w}")
                nc.tensor.matmul(warm, ones_cb, ones_s, start=True, stop=True,
                                 skip_group_check=True)

            # ---- span coverage counts ------------------------------------
            # covered = ge(iota,s) - ge(iota,e); the subtraction rides the
            # PE accumulation via signed host onehots (+oh for s, -oh for e),
            # so no rank-1 correction is needed.
            counts_ps = ps_cnt.tile([T, S], F32, tag="cnt")
            for i in range(NT):
                ge_s = masks.tile([P, S], BF16, tag="ge_s")
                nc.vector.tensor_scalar(
                    out=ge_s, in0=iota16,
                    scalar1=cf32_sb[:, O_M + i:O_M + i + 1],
                    scalar2=None, op0=ALU.is_ge,
                )
                ge_e = masks.tile([P, S], BF16, tag="ge_e")
                nc.vector.tensor_scalar(
                    out=ge_e, in0=iota16,
                    scalar1=cf32_sb[:, O_R + i:O_R + i + 1],
                    scalar2=None, op0=ALU.is_ge,
                )
                nc.tensor.matmul(
                    counts_ps,
                    cf16_sb[:, S + 2 * i * T:S + (2 * i + 1) * T], ge_s,
                    start=(i == 0), stop=False, skip_group_check=True,
                )
                nc.tensor.matmul(
                    counts_ps,
                    cf16_sb[:, S + (2 * i + 1) * T:S + (2 * i + 2) * T], ge_e,
                    start=False, stop=(i == NT - 1), skip_group_check=True,
                )

            # counts -> f8 (ints, exact); slot 1 stays zero so the DR pass
            # contracts only slot 0 against weffp8.
            for h0, h1 in ((0, HS), (HS, S)):
                nc.scalar.activation(
                    out=counts8[0:T, 0:1, h0:h1], in_=counts_ps[:, h0:h1],
                    func=AF.Identity, bias=zcol[0:T, :], scale=1.0,
                )

            # ---- h1 = relu(W_eff.T @ counts + b): fp8 DR, half-width ------
            r8 = singles.tile([P, KC_H, S], F8)
            relu_eng = 0
            for jc in range(KC_H):
                h1p = ps_big.tile([P, S], F32, tag="big", name=f"h1_{jc}")
                for h0, h1 in ((0, HS), (HS, S)):
                    nc.tensor.matmul(
                        h1p[:, h0:h1],
                        wf8_sb[:, :, jc * P:(jc + 1) * P],
                        counts8[:, :, h0:h1],
                        start=(h0 == 0), stop=(h0 != 0),
                        perf_mode=DR, skip_group_check=True,
                    )
                for h0, h1 in ((0, HS), (HS, S)):
                    e = relu_eng % 2
                    relu_eng += 1
                    if e == 0:
                        nc.vector.tensor_scalar(
                            out=r8[:, jc, h0:h1], in0=h1p[:, h0:h1],
                            scalar1=RSCL, scalar2=0.0,
                            op0=ALU.mult, op1=ALU.max,
                        )
                    else:
                        nc.scalar.activation(
                            out=r8[:, jc, h0:h1], in_=h1p[:, h0:h1],
                            func=AF.Relu, bias=zcol, scale=RSCL,
                        )

            # ---- pr psum: raw rows 0:33, sum row 64 -----------------------
            pr = ps_pr.tile([MPR, S], F32, tag="pr")
            sqs = []
            for fc in range(KC_H):
                nc.tensor.matmul(
                    pr[0:MPR, :],
                    cbf_sb[:, fc * MPR:(fc + 1) * MPR],
                    we_sb[:, fc, :],
                    start=(fc == 0), stop=False,
                    skip_group_check=True,
                )
                sq = work.tile([P, S], BF16, tag="sq", name=f"sqwe{fc}")
                if fc < 4:
                    nc.vector.tensor_mul(
                        out=sq, in0=we_sb[:, fc, :], in1=we_sb[:, fc, :])
                else:
                    nc.gpsimd.tensor_tensor(
                        out=sq, in0=we_sb[:, fc, :], in1=we_sb[:, fc, :],
                        op=ALU.mult)
                sqs.append(sq)

            # G part of raw/sum: 3 fp8 DR matmuls into the same psum
            for pp in range(KC_H2):
                nc.tensor.matmul(
                    pr[0:MPR, :],
                    cf8_sb[:, pp, :, H2:H2 + MPR],
                    r8[:, 2 * pp:2 * pp + 2, :],
                    start=False, stop=False,
                    perf_mode=DR, skip_group_check=True,
                )

            # mu = pr[64]/NEW_H + sum_ff2b/NEW_H  (row 1 of mu_sb is ones)
            for h0, h1 in ((0, HS), (HS, S)):
                nc.vector.tensor_scalar(
                    out=mu_sb[0:1, h0:h1], in0=pr[MPR - 1:MPR, h0:h1],
                    scalar1=1.0 / NEW_H,
                    scalar2=cf32_sb[0:1, O_MUB:O_MUB + 1],
                    op0=ALU.mult, op1=ALU.add,
                )
            mu2 = singles.tile([1, S], BF16)
            for h0, h1 in ((0, HS), (HS, S)):
                nc.vector.tensor_mul(
                    out=mu2[:, h0:h1], in0=mu_sb[0:1, h0:h1],
                    in1=mu_sb[0:1, h0:h1])

            # rank-1 [c1n; d] @ [mu; ones] fold closes the pr group
            nc.tensor.matmul(
                pr[0:NL, :], cbf_sb[0:2, OFF_W:OFF_W + NL], mu_sb,
                start=False, stop=True, skip_group_check=True,
            )
            praw = singles.tile([NL, S], BF16)
            nc.vector.tensor_copy(out=praw[:, 0:HS], in_=pr[0:NL, 0:HS])
            nc.scalar.activation(
                out=praw[:, HS:S], in_=pr[0:NL, HS:S],
                func=AF.Identity, bias=zcol[0:NL, :], scale=1.0,
            )

            # ---- h2 chunks (fp8 DR) + squares -----------------------------
            ss = ps_ss.tile([1, S], F32, tag="ss")
            sqh = []
            for mc in range(KC_H2):
                h2p = ps_big.tile([P, S], F32, tag="big", name=f"h2_{mc}")
                for h0, h1 in ((0, HS), (HS, S)):
                    for pp in range(KC_H2):
                        nc.tensor.matmul(
                            h2p[:, h0:h1],
                            cf8_sb[:, pp, :, mc * P:(mc + 1) * P],
                            r8[:, 2 * pp:2 * pp + 2, h0:h1],
                            start=(pp == 0 and h0 == 0),
                            stop=(pp == KC_H2 - 1 and h0 != 0),
                            perf_mode=DR, skip_group_check=True,
                        )
                for h0, h1 in ((0, HS), (HS, S)):
                    sq = work.tile([P, HS], BF16, tag="sqh",
                                   name=f"sqh2_{mc}_{h0}")
                    if mc == 2:
                        y = work.tile([P, HS], BF16, tag="sqh",
                                      name=f"y_{mc}_{h0}")
                        nc.vector.tensor_scalar(
                            out=y, in0=h2p[:, h0:h1], scalar1=GSC / FSC,
                            scalar2=cf32_sb[:, O_F2B + mc:O_F2B + mc + 1],
                            op0=ALU.mult, op1=ALU.add,
                        )
                        nc.vector.tensor_mul(out=sq, in0=y, in1=y)
                    else:
                        nc.scalar.activation(
                            out=sq, in_=h2p[:, h0:h1], func=AF.Square,
                            bias=cf32_sb[:, O_F2B + mc:O_F2B + mc + 1],
                            scale=GSC / FSC,
                        )
                    sqh.append((sq, h0, h1))

            # ---- ss: sumsq/NEW_H row, independent column-half groups ------
            # start=True zero-marks the whole 2KB psum zero-region, so only
            # the very first pass may carry it; later passes on untouched
            # bytes lazily zero via the pending-zero consumption.
            for h0, h1 in ((0, HS), (HS, S)):
                for fc in range(KC_H):
                    nc.tensor.matmul(
                        ss[:, h0:h1], inv_col, sqs[fc][:, h0:h1],
                        start=(fc == 0 and h0 == 0), stop=False,
                        skip_group_check=True,
                    )
            for sq, h0, h1 in sqh:
                nc.tensor.matmul(
                    ss[:, h0:h1], inv_col, sq,
                    start=False, stop=False, skip_group_check=True,
                )
            # -mu^2 closes each half: ss then holds var = E[x^2] - mu^2
            for h0, h1 in ((0, HS), (HS, S)):
                nc.tensor.matmul(
                    ss[:, h0:h1], neg1, mu2[:, h0:h1],
                    start=False, stop=(h0 != 0), skip_group_check=True,
                )

            # ---- rstd + output --------------------------------------------
            sd = singles.tile([1, S], F32)
            rstd = singles.tile([1, S], BF16)
            rb = ps_rb.tile([NL, S], F32, tag="rb")
            f_sb = singles.tile([NL, S], F32)
            t2 = singles.tile([NL, S], BF16)
            for h0, h1 in ((0, HS), (HS, S)):
                nc.scalar.activation(
                    out=sd[:, h0:h1], in_=ss[:, h0:h1], func=AF.Sqrt,
                    bias=eps_t, scale=1.0,
                )
                with nc.allow_low_precision(reason="bf16 rstd ample for LN"):
                    nc.vector.reciprocal(
                        out=rstd[:, h0:h1], in_=sd[:, h0:h1])
                nc.tensor.matmul(
                    rb[:, h0:h1], ones_row, rstd[:, h0:h1],
                    start=True, stop=True, skip_group_check=True,
                )
                nc.vector.tensor_mul(
                    out=t2[:, h0:h1], in0=praw[:, h0:h1], in1=rb[:, h0:h1])
                nc.vector.tensor_scalar(
                    out=f_sb[:, h0:h1], in0=t2[:, h0:h1],
                    scalar1=cf32_sb[0:NL, O_C2:O_C2 + 1],
                    scalar2=None, op0=ALU.add,
                )
            nc.sync.dma_start(out=out[:, 0:HS], in_=f_sb[:, 0:HS])
            nc.scalar.dma_start(out=out[:, HS:S], in_=f_sb[:, HS:S])

    nc.compile()
    return nc


_CACHE = {}


def kernel(**inputs) -> np.ndarray:
    bfl = ml_dtypes.bfloat16
    f8 = ml_dtypes.float8_e4m3
    we = np.asarray(inputs["word_embedding"], np.float32)
    te = np.asarray(inputs["tag_embedding"], np.float32)
    ipw = np.asarray(inputs["in_proj_w"], np.float32)
    ipb = np.asarray(inputs["in_proj_b"], np.float32)
    opw = np.asarray(inputs["out_proj_w"], np.float32)
    ob_ = np.asarray(inputs["out_proj_b"], np.float32)
    f1w = np.asarray(inputs["ff1_w"], np.float32)
    f1b = np.asarray(inputs["ff1_b"], np.float32)
    f2w = np.asarray(inputs["ff2_w"], np.float32)
    f2b = np.asarray(inputs["ff2_b"], np.float32)
    lg = np.asarray(inputs["ln_g"], np.float32)
    lb = np.asarray(inputs["ln_b"], np.float32)
    lw = np.asarray(inputs["lin_w"], np.float32)
    lbias = np.asarray(inputs["lin_b"], np.float32)
    sb = np.asarray(inputs["span_batch"]).astype(np.int64)
    st = np.asarray(inputs["span_tag"]).astype(np.int64)
    ss = np.asarray(inputs["span_start"]).astype(np.int64)
    se = np.asarray(inputs["span_end"]).astype(np.int64)

    # ---- host-side weight folding -------------------------------------
    v_tag = (te @ ipw[2 * H:].T + ipb[2 * H:]) @ opw.T + ob_       # [T, H]
    w_eff = np.einsum("th,jth->tj", v_tag, f1w.reshape(H, T, H))   # [T, H]
    weffp = np.concatenate([w_eff, f1b[None, :]], 0)               # [17, H]
    lwg = lw.T * lg[:, None]                                       # [NEW_H, NL]
    lwg_we, lwg_h2 = lwg[:H], lwg[H:]
    g_plus = np.zeros((H, 65), np.float32)                         # [H, 65]
    g_plus[:, :NL] = f2w.T @ lwg_h2
    g_plus[:, 64] = f2w.sum(0)
    g_plus *= GSC
    c1n = -lwg.sum(0)                                              # [NL]
    d = lwg_h2.T @ f2b                                             # [NL]
    c2 = lw @ lb + lbias                                           # [NL]
    sum_ff2b = float(f2b.sum())

    counts_per_b = np.bincount(sb, minlength=B)
    NT = max(1, int(np.ceil(counts_per_b.max() / P)))
    n_pad = NT * P

    # cbf: [128, 6*65 lwgwe | NL [c1n;d] cols]
    OFF_W = KC_H * MPR
    CBF = OFF_W + NL
    cbf = np.zeros((P, CBF), np.float32)
    lwgwe_plus = np.zeros((H, MPR), np.float32)                    # [H, 65]
    lwgwe_plus[:, :NL] = lwg_we
    lwgwe_plus[:, 64] = 1.0
    cbf[:, :OFF_W] = lwgwe_plus.reshape(KC_H, P, MPR).transpose(
        1, 0, 2).reshape(P, OFF_W)
    cbf[0, OFF_W:OFF_W + NL] = c1n
    cbf[1, OFF_W:OFF_W + NL] = d
    cbf = cbf.astype(bfl)

    # cf8: per K-pair pp: [ff2t8(384) | g65 | pad] width 512
    cf8 = np.zeros((P, KC_H2, 2, W8), np.float32)
    ff2t8 = (f2w.T * FSC).reshape(KC_H2, 2, P, H2)                 # [3,2,128,H2]
    g16p = g_plus.reshape(KC_H2, 2, P, MPR)                        # [3,2,128,65]
    cf8[:, :, :, 0:H2] = ff2t8.transpose(2, 0, 1, 3)
    cf8[:, :, :, H2:H2 + MPR] = g16p.transpose(2, 0, 1, 3)
    cf8 = cf8.astype(f8)

    # wf8: [16, 2, H]: slot 0 = w_eff * W8SC, slot 1 row 0 = f1b * W8SC
    wf8 = np.zeros((T, 2, H), np.float32)
    wf8[:, 0, :] = w_eff * W8SC
    wf8[0, 1, :] = f1b * W8SC
    wf8 = wf8.astype(f8)

    CF16 = S + 2 * NT * T
    CF32 = 2 * NT + KC_H2 + 2
    O_F2B = 2 * NT
    base32 = np.zeros((P, CF32), np.float32)
    base32[:, O_F2B:O_F2B + KC_H2] = f2b.reshape(KC_H2, P).T
    base32[0, O_F2B + KC_H2] = sum_ff2b / NEW_H
    base32[0:NL, O_F2B + KC_H2 + 1] = c2

    in_maps = []
    for c in range(NCORES):
        idx = np.where(sb == c)[0]
        n = len(idx)
        spsv = np.zeros(n_pad, np.float32)
        spev = np.zeros(n_pad, np.float32)
        sptv = np.full(n_pad, -1, np.int64)   # pad tag -1: no onehot row
        spsv[:n] = ss[idx]
        spev[:n] = se[idx]
        sptv[:n] = st[idx]
        spsv[n:] = S + 64                     # pad: ge never true
        spev[n:] = S + 64
        cf32_c = base32.copy()
        cf32_c[:, 0:NT] = spsv.reshape(NT, P).T
        cf32_c[:, NT:2 * NT] = spev.reshape(NT, P).T
        cf16_c = np.zeros((P, CF16), np.float16)
        cf16_c[:, :S] = np.arange(S, dtype=np.float16)
        oh = np.zeros((NT * P, 2, T), np.float32)
        valid = sptv >= 0
        rows = np.arange(NT * P)[valid]
        oh[rows, 0, sptv[valid]] = 1.0
        oh[rows, 1, sptv[valid]] = -1.0
        cf16_c[:, S:] = oh.reshape(NT, P, 2 * T).transpose(1, 0, 2).reshape(
            P, 2 * NT * T).astype(np.float16)
        we_c = np.ascontiguousarray(we[c].T).reshape(KC_H, P, S)
        in_maps.append(dict(
            cf16=cf16_c,
            cf32=cf32_c,
            cbf=cbf,
            cf8=cf8,
            wf8=wf8,
            we_t=np.ascontiguousarray(
                we_c.transpose(1, 0, 2)).astype(bfl),
        ))

    if NT not in _CACHE:
        _CACHE[NT] = build_kernel(NT)
    nc = _CACHE[NT]

    res = run_bass_kernel_spmd(nc, in_maps, list(range(NCORES)))
    outv = np.stack([res.results[c]["out"].T for c in range(NCORES)])
    return outv.astype(np.float32)


if __name__ == "__main__":
    import reference
    inp = {k: np.asarray(v) for k, v in reference.setup_inputs().items()}
    got = kernel(**inp)
    print("kernel output:", got.shape, got.dtype)
